# revision 5
# baseline (speedup 1.0000x reference)
"""LIF Conv2d layer (one timestep) on 8 Trainium2 NeuronCores.

Data-parallel over batch: 64 images -> 8 per core. Each core:
  P_new = ALPHA*P + (1-ALPHA)*Q     (folded: pb = P + cq*Q, weights pre-scaled by ALPHA)
  U     = conv2d(P_new, W, bias) - GAMMA*R
  S     = (U > 1).float
  readout = (maxpool2x2(S).flat @ w_sign.T + 1) * 0.5

Conv is 25 shifted matmuls per 8-row tile, tap-paired into 15 via a
+1-column-shifted duplicate of the padded image on SBUF partitions 64..127
(K = 2 taps x 64 Cin = 128). PSUM accumulates the 5x5 window; epilogue on
DVE/ACT computes U and S; maxpool + sign-readout matmuls finish on-chip.

Self-contained: hardcodes all shapes; only needs numpy + concourse (+ jax via
axon for device execution inside run_bass_kernel_spmd).
"""

import os
from contextlib import ExitStack

import numpy as np
import ml_dtypes

import concourse.bass as bass
import concourse.tile as tile
from concourse import bacc, mybir
from concourse.bass_utils import run_bass_kernel_spmd

# ---------------------------------------------------------------- constants
N_CORES = 8
B, CIN, COUT, H, W, KK = 64, 64, 128, 64, 64, 5
HW = H * W              # 4096
BPC = B // N_CORES      # images per core
PW = W + 4              # padded row width (2 left + 2 right)
PBN = H * PW            # padded image elements per partition
ROWT = 8                # output rows per conv tile
NTILE = H // ROWT       # row tiles per image
NPOOL = (H // 2) * (W // 2)   # 1024 pooled pixels per channel
NOUT = 10

ALPHA = float(np.exp(-1e-3 / 20e-3))
GAMMA = float(np.exp(-1e-3 / 2.86e-3))
CQ = (1.0 - ALPHA) / ALPHA
THR = 1.0

# conv matmul dtype: "f32r" (full speed, reduced-precision HW multiply),
# "f32" (exact, 4x slower), "bf16" (fast, lossy)
CONV_DTYPE = os.environ.get("LIF_CONV_DTYPE", "f32r")

# tap schedule: (dy, dxa, is_pair). Pairs cover (dy,dxa)+(dy,dxa+1).
# First entry must fully cover the output tile (dy=2 -> no row clipping).
_TAPS = [(2, 2, True)] + [
    (dy, dxa, p)
    for dy in range(KK)
    for (dxa, p) in ((0, True), (2, True), (4, False))
    if (dy, dxa, p) != (2, 2, True)
]
assert len(_TAPS) == 15


def _row_clip(t, dy):
    """Valid output-row subrange [lo, hi) within tile t for tap row dy."""
    lo = max(0, 2 - dy) if t == 0 else 0
    hi = ROWT - (max(0, dy - 2) if t == NTILE - 1 else 0)
    return lo, hi


def _build(n_img=BPC):
    f32 = mybir.dt.float32
    bf16 = mybir.dt.bfloat16
    conv_dt = {"f32r": mybir.dt.float32r, "f32": f32, "bf16": bf16}[CONV_DTYPE]
    # pb / wconv tiles carry the conv dtype natively: the BIR verifier
    # requires every producer feeding an fp32r matmul to emit fp32r.
    sb_dt = conv_dt

    nc = bacc.Bacc("TRN2", target_bir_lowering=False, debug=False)

    p_in = nc.dram_tensor("p_in", [n_img, CIN, HW], f32, kind="ExternalInput")
    q_in = nc.dram_tensor("q_in", [n_img, CIN, HW], f32, kind="ExternalInput")
    r_in = nc.dram_tensor("r_in", [n_img, COUT, HW], f32, kind="ExternalInput")
    wconv = nc.dram_tensor("wconv", [128, 15 * 128], sb_dt, kind="ExternalInput")
    bias_v = nc.dram_tensor("bias_v", [COUT, 1], f32, kind="ExternalInput")
    wsT = nc.dram_tensor("wsT", [128, NPOOL * NOUT], bf16, kind="ExternalInput")

    s_out = nc.dram_tensor("s_out", [n_img, COUT, HW], f32, kind="ExternalOutput")
    u_out = nc.dram_tensor("u_out", [n_img, COUT, HW], f32, kind="ExternalOutput")
    ro_out = nc.dram_tensor("ro_out", [n_img, NOUT], f32, kind="ExternalOutput")

    with tile.TileContext(nc) as tc, ExitStack() as ctx:
        cpool = ctx.enter_context(tc.tile_pool(name="const", bufs=1))
        iopool = ctx.enter_context(tc.tile_pool(name="io", bufs=1))
        rpool = ctx.enter_context(tc.tile_pool(name="r", bufs=2))
        wpool = ctx.enter_context(tc.tile_pool(name="work", bufs=4))
        pbpool = ctx.enter_context(tc.tile_pool(name="pb", bufs=1))
        pspool = ctx.enter_context(
            tc.tile_pool(name="psum", bufs=4, space="PSUM")
        )
        ropool = ctx.enter_context(
            tc.tile_pool(name="ropsum", bufs=1, space="PSUM")
        )

        # constants
        wc_sb = cpool.tile([128, 15 * 128], sb_dt, tag="wc")
        nc.sync.dma_start(wc_sb[:], wconv.ap())
        ws_sb = cpool.tile([128, NPOOL * NOUT], bf16, tag="ws")
        nc.sync.dma_start(ws_sb[:], wsT.ap())
        bias_sb = cpool.tile([COUT, 1], f32, tag="bias")
        nc.sync.dma_start(bias_sb[:], bias_v.ap())
        pooled = cpool.tile([128, NPOOL * n_img], bf16, tag="pooled")

        wc3 = wc_sb[:].rearrange("p (k o) -> p k o", o=128)

        # two persistent padded-image buffers, ping-ponged across images
        pb_tiles = [
            pbpool.tile([128, PBN], sb_dt, tag=f"pb{i}", name=f"pb{i}")
            for i in range(2)
        ]
        for pb in pb_tiles:
            pbz = pb[:].bitcast(mybir.dt.int32).rearrange("p (r c) -> p r c", c=PW)
            nc.gpsimd.memset(pbz[0:CIN, :, 0:2], 0)
            nc.gpsimd.memset(pbz[0:CIN, :, W + 2 : PW], 0)

        for b in range(n_img):
            pb = pb_tiles[b % 2]
            pb3 = pb[:].rearrange("p (r c) -> p r c", c=PW)

            p_raw = iopool.tile([CIN, HW], f32, tag="p_raw")
            nc.sync.dma_start(p_raw[:], p_in.ap()[b])
            q_raw = iopool.tile([CIN, HW], f32, tag="q_raw")
            nc.sync.dma_start(q_raw[:], q_in.ap()[b])
            r_raw = rpool.tile([COUT, HW], f32, tag="r_raw")
            nc.sync.dma_start(r_raw[:], r_in.ap()[b])

            # pb interior = Q*cq + P   (P_new / ALPHA)
            nc.vector.scalar_tensor_tensor(
                pb3[0:CIN, :, 2 : W + 2],
                q_raw[:].rearrange("p (r c) -> p r c", c=W),
                CQ,
                p_raw[:].rearrange("p (r c) -> p r c", c=W),
                op0=mybir.AluOpType.mult,
                op1=mybir.AluOpType.add,
            )
            # +1-column-shifted duplicate on partitions 64..127
            nc.sync.dma_start(pb[CIN:128, 0 : PBN - 1], pb[0:CIN, 1:PBN])

            r3 = r_raw[:].rearrange("p (t n) -> p t n", n=ROWT * W)
            u3 = u_out.ap()[b].rearrange("p (t n) -> p t n", n=ROWT * W)
            s3 = s_out.ap()[b].rearrange("p (t n) -> p t n", n=ROWT * W)

            for t in range(NTILE):
                # rs = -GAMMA * R  (ScalarE, also stages PSUM-adjacent operand)
                rs = wpool.tile([COUT, ROWT * W], f32, tag="rs")
                nc.scalar.activation(
                    rs[:], r3[:, t], mybir.ActivationFunctionType.Copy,
                    bias=0.0, scale=-GAMMA,
                )

                ps = pspool.tile([128, ROWT * W], f32, tag="ps")
                psr = ps[:].rearrange("p (r c) -> p r c", c=W)
                for i, (dy, dxa, is_pair) in enumerate(_TAPS):
                    lo, hi = _row_clip(t, dy)
                    r0 = t * ROWT + lo + dy - 2
                    kmax = 128 if is_pair else CIN
                    rhs = pb3[0:kmax, r0 : r0 + (hi - lo), dxa : dxa + W]
                    lhsT = wc3[0:kmax, i, :]
                    if CONV_DTYPE == "f32r":
                        rhs = rhs.bitcast(mybir.dt.float32r)
                        lhsT = lhsT.bitcast(mybir.dt.float32r)
                    nc.tensor.matmul(
                        psr[:, lo:hi, :], lhsT, rhs,
                        start=(i == 0), stop=(i == len(_TAPS) - 1),
                    )

                # U = (psum + bias) + rs ; S = U > 1
                u_sb = wpool.tile([COUT, ROWT * W], f32, tag="u_sb")
                nc.vector.scalar_tensor_tensor(
                    u_sb[:], ps[:], bias_sb[:], rs[:],
                    op0=mybir.AluOpType.add, op1=mybir.AluOpType.add,
                )
                s_sb = wpool.tile([COUT, ROWT * W], f32, tag="s_sb")
                nc.vector.tensor_scalar(
                    s_sb[:], u_sb[:], THR, None, op0=mybir.AluOpType.is_gt
                )
                nc.sync.dma_start(u3[:, t], u_sb[:])
                nc.sync.dma_start(s3[:, t], s_sb[:])

                # maxpool 2x2: row pairs then column pairs
                sq = s_sb[:].rearrange("p (r2 two c) -> p r2 two c", two=2, c=W)
                a_t = wpool.tile([128, (ROWT // 2) * W], f32, tag="a_t")
                a3 = a_t[:].rearrange("p (r c2 two) -> p r c2 two", two=2, c2=W // 2)
                nc.vector.tensor_tensor(
                    a_t[:], sq[:, :, 0, :], sq[:, :, 1, :], op=mybir.AluOpType.max
                )
                po5 = pooled[:].rearrange(
                    "p (tt pr pc b) -> p tt pr pc b",
                    tt=NTILE, pr=ROWT // 2, pc=W // 2,
                )
                nc.vector.tensor_tensor(
                    po5[:, t, :, :, b], a3[:, :, :, 0], a3[:, :, :, 1],
                    op=mybir.AluOpType.max,
                )

        # readout: accumulate over all 1024 p-chunks of K=128 (channels)
        ro_ps = ropool.tile([n_img, NOUT], f32, tag="rops")
        pog = pooled[:].rearrange("p (q b) -> p q b", b=n_img)
        wsg = ws_sb[:].rearrange("p (q o) -> p q o", o=NOUT)
        for p in range(NPOOL):
            nc.tensor.matmul(
                ro_ps[:], pog[:, p, :], wsg[:, p, :],
                start=(p == 0), stop=(p == NPOOL - 1),
            )
        ro_sb = cpool.tile([n_img, NOUT], f32, tag="rosb")
        nc.scalar.activation(
            ro_sb[:], ro_ps[:], mybir.ActivationFunctionType.Copy,
            bias=0.5, scale=0.5,
        )
        nc.sync.dma_start(ro_out.ap(), ro_sb[:])

    nc.compile()
    return nc


_NC_CACHE = {}


def _get_nc(n_img=BPC):
    key = (n_img, CONV_DTYPE)
    if key not in _NC_CACHE:
        _NC_CACHE[key] = _build(n_img)
    return _NC_CACHE[key]


def _host_prep(weights, bias, w_sign):
    sb_np = ml_dtypes.bfloat16 if CONV_DTYPE == "bf16" else np.float32
    wpre = (ALPHA * weights).astype(np.float32)  # [COUT, CIN, 5, 5]
    wconv = np.zeros((128, 15, 128), dtype=np.float32)
    for k, (dy, dxa, is_pair) in enumerate(_TAPS):
        wconv[0:CIN, k, :] = wpre[:, :, dy, dxa].T
        if is_pair:
            wconv[CIN:128, k, :] = wpre[:, :, dy, dxa + 1].T
    wconv = np.ascontiguousarray(wconv.reshape(128, 15 * 128).astype(sb_np))
    bias_v = np.ascontiguousarray(bias.reshape(COUT, 1).astype(np.float32))
    wsT = np.ascontiguousarray(
        w_sign.reshape(NOUT, 128, NPOOL).transpose(1, 2, 0)
        .reshape(128, NPOOL * NOUT).astype(ml_dtypes.bfloat16)
    )
    return wconv, bias_v, wsT


LAST_RESULT = None


def kernel(input_t=None, P=None, Q=None, R=None, weights=None, bias=None,
           w_sign=None, **_unused):
    global LAST_RESULT
    P = np.asarray(P, dtype=np.float32)
    Q = np.asarray(Q, dtype=np.float32)
    R = np.asarray(R, dtype=np.float32)
    weights = np.asarray(weights, dtype=np.float32)
    bias = np.asarray(bias, dtype=np.float32)
    w_sign = np.asarray(w_sign, dtype=np.float32)

    nc = _get_nc()
    wconv, bias_v, wsT = _host_prep(weights, bias, w_sign)

    in_maps = []
    for c in range(N_CORES):
        sl = slice(c * BPC, (c + 1) * BPC)
        in_maps.append({
            "p_in": np.ascontiguousarray(P[sl].reshape(BPC, CIN, HW)),
            "q_in": np.ascontiguousarray(Q[sl].reshape(BPC, CIN, HW)),
            "r_in": np.ascontiguousarray(R[sl].reshape(BPC, COUT, HW)),
            "wconv": wconv,
            "bias_v": bias_v,
            "wsT": wsT,
        })

    trace = bool(int(os.environ.get("LIF_TRACE", "0")))
    res = run_bass_kernel_spmd(nc, in_maps, core_ids=list(range(N_CORES)),
                               trace=trace)
    LAST_RESULT = res

    S = np.concatenate([r["s_out"] for r in res.results], axis=0)
    U = np.concatenate([r["u_out"] for r in res.results], axis=0)
    RO = np.concatenate([r["ro_out"] for r in res.results], axis=0)
    return (
        S.reshape(B, COUT, H, W),
        U.reshape(B, COUT, H, W),
        RO.reshape(B, NOUT),
    )


# revision 8
# speedup vs baseline: 1.7373x; 1.7373x over previous
"""LIF Conv2d layer (one timestep) on 8 Trainium2 NeuronCores.

Data-parallel over batch: 64 images -> 8 per core. Each core:
  P_new = ALPHA*P + (1-ALPHA)*Q     (folded: pb = P + cq*Q, weights pre-scaled by ALPHA)
  U     = conv2d(P_new, W, bias) - GAMMA*R
  S     = (U > 1).float
  readout = (maxpool2x2(S).flat @ w_sign.T + 1) * 0.5

Conv is 25 shifted matmuls per 8-row tile, tap-paired into 15 via a
+1-column-shifted duplicate of the padded image on SBUF partitions 64..127
(K = 2 taps x 64 Cin = 128). PSUM accumulates the 5x5 window; epilogue on
DVE/ACT computes U and S; maxpool + sign-readout matmuls finish on-chip.

Self-contained: hardcodes all shapes; only needs numpy + concourse (+ jax via
axon for device execution inside run_bass_kernel_spmd).
"""

import os
from contextlib import ExitStack

import numpy as np
import ml_dtypes

import concourse.bass as bass
import concourse.tile as tile
from concourse import bacc, mybir
from concourse.bass_utils import run_bass_kernel_spmd

# ---------------------------------------------------------------- constants
N_CORES = 8
B, CIN, COUT, H, W, KK = 64, 64, 128, 64, 64, 5
HW = H * W              # 4096
BPC = B // N_CORES      # images per core
PW = W + 4              # padded row width (2 left + 2 right)
PBN = H * PW            # padded image elements per partition
ROWT = 8                # output rows per conv tile
NTILE = H // ROWT       # row tiles per image
NPOOL = (H // 2) * (W // 2)   # 1024 pooled pixels per channel
NOUT = 10

ALPHA = float(np.exp(-1e-3 / 20e-3))
GAMMA = float(np.exp(-1e-3 / 2.86e-3))
CQ = (1.0 - ALPHA) / ALPHA
THR = 1.0

# conv matmul dtype: "f32r" (full speed, reduced-precision HW multiply),
# "f32" (exact, 4x slower), "bf16" (fast, lossy)
CONV_DTYPE = os.environ.get("LIF_CONV_DTYPE", "f32r")

# tap schedule: (dy, dxa, is_pair). Pairs cover (dy,dxa)+(dy,dxa+1).
# First entry must fully cover the output tile (dy=2 -> no row clipping).
_TAPS = [(2, 2, True)] + [
    (dy, dxa, p)
    for dy in range(KK)
    for (dxa, p) in ((0, True), (2, True), (4, False))
    if (dy, dxa, p) != (2, 2, True)
]
assert len(_TAPS) == 15


def _row_clip(t, dy):
    """Valid output-row subrange [lo, hi) within tile t for tap row dy."""
    lo = max(0, 2 - dy) if t == 0 else 0
    hi = ROWT - (max(0, dy - 2) if t == NTILE - 1 else 0)
    return lo, hi


def _build(n_img=BPC):
    f32 = mybir.dt.float32
    bf16 = mybir.dt.bfloat16
    conv_dt = {"f32r": mybir.dt.float32r, "f32": f32, "bf16": bf16}[CONV_DTYPE]
    # pb / wconv tiles carry the conv dtype natively: the BIR verifier
    # requires every producer feeding an fp32r matmul to emit fp32r.
    sb_dt = conv_dt

    nc = bacc.Bacc("TRN2", target_bir_lowering=False, debug=False)

    p_in = nc.dram_tensor("p_in", [n_img, CIN, HW], f32, kind="ExternalInput")
    q_in = nc.dram_tensor("q_in", [n_img, CIN, HW], f32, kind="ExternalInput")
    r_in = nc.dram_tensor("r_in", [n_img, COUT, HW], f32, kind="ExternalInput")
    wconv = nc.dram_tensor("wconv", [128, 15 * 128], sb_dt, kind="ExternalInput")
    bias_v = nc.dram_tensor("bias_v", [COUT, 1], f32, kind="ExternalInput")
    wsT = nc.dram_tensor("wsT", [128, NPOOL * NOUT], bf16, kind="ExternalInput")

    s_out = nc.dram_tensor("s_out", [n_img, COUT, HW], f32, kind="ExternalOutput")
    u_out = nc.dram_tensor("u_out", [n_img, COUT, HW], f32, kind="ExternalOutput")
    ro_out = nc.dram_tensor("ro_out", [n_img, NOUT], f32, kind="ExternalOutput")

    with tile.TileContext(nc) as tc, ExitStack() as ctx:
        cpool = ctx.enter_context(tc.tile_pool(name="const", bufs=1))
        iopool = ctx.enter_context(tc.tile_pool(name="io", bufs=1))
        rpool = ctx.enter_context(tc.tile_pool(name="r", bufs=2))
        wpool = ctx.enter_context(tc.tile_pool(name="work", bufs=4))
        pbpool = ctx.enter_context(tc.tile_pool(name="pb", bufs=1))
        pspool = ctx.enter_context(
            tc.tile_pool(name="psum", bufs=8, space="PSUM")
        )

        # constants
        wc_sb = cpool.tile([128, 15 * 128], sb_dt, tag="wc")
        nc.sync.dma_start(wc_sb[:], wconv.ap())
        ws_sb = cpool.tile([128, NPOOL * NOUT], bf16, tag="ws")
        nc.sync.dma_start(ws_sb[:], wsT.ap())
        bias_sb = cpool.tile([COUT, 1], f32, tag="bias")
        nc.sync.dma_start(bias_sb[:], bias_v.ap())
        pooled = cpool.tile([128, NPOOL * n_img], bf16, tag="pooled")

        wc3 = wc_sb[:].rearrange("p (k o) -> p k o", o=128)

        # two persistent padded-image buffers, ping-ponged across images
        pb_tiles = [
            pbpool.tile([128, PBN], sb_dt, tag=f"pb{i}", name=f"pb{i}")
            for i in range(2)
        ]
        for pb in pb_tiles:
            pbz = pb[:].bitcast(mybir.dt.int32).rearrange("p (r c) -> p r c", c=PW)
            nc.gpsimd.memset(pbz[0:CIN, :, 0:2], 0)
            nc.gpsimd.memset(pbz[0:CIN, :, W + 2 : PW], 0)

        for b in range(n_img):
            pb = pb_tiles[b % 2]
            pb3 = pb[:].rearrange("p (r c) -> p r c", c=PW)

            p_raw = iopool.tile([CIN, HW], f32, tag="p_raw")
            nc.sync.dma_start(p_raw[:], p_in.ap()[b])
            q_raw = iopool.tile([CIN, HW], f32, tag="q_raw")
            nc.sync.dma_start(q_raw[:], q_in.ap()[b])
            r_raw = rpool.tile([COUT, HW], f32, tag="r_raw")
            nc.sync.dma_start(r_raw[:], r_in.ap()[b])

            # pb interior = Q*cq + P   (P_new / ALPHA)
            nc.vector.scalar_tensor_tensor(
                pb3[0:CIN, :, 2 : W + 2],
                q_raw[:].rearrange("p (r c) -> p r c", c=W),
                CQ,
                p_raw[:].rearrange("p (r c) -> p r c", c=W),
                op0=mybir.AluOpType.mult,
                op1=mybir.AluOpType.add,
            )
            # +1-column-shifted duplicate on partitions 64..127
            nc.sync.dma_start(pb[CIN:128, 0 : PBN - 1], pb[0:CIN, 1:PBN])

            r3 = r_raw[:].rearrange("p (t n) -> p t n", n=ROWT * W)
            u3 = u_out.ap()[b].rearrange("p (t n) -> p t n", n=ROWT * W)
            s3 = s_out.ap()[b].rearrange("p (t n) -> p t n", n=ROWT * W)

            # tap-outer over all 8 row tiles: one stationary weight serves an
            # 8-matmul burst (the fp32r self-loading weight reload otherwise
            # halves the rhs stream rate on every matmul)
            pss = [
                pspool.tile([128, ROWT * W], f32, tag="ps", name=f"ps_{b}_{t}")
                for t in range(NTILE)
            ]
            for i, (dy, dxa, is_pair) in enumerate(_TAPS):
                kmax = 128 if is_pair else CIN
                lhsT = wc3[0:kmax, i, :]
                if CONV_DTYPE == "f32r":
                    lhsT = lhsT.bitcast(mybir.dt.float32r)
                for t in range(NTILE):
                    lo, hi = _row_clip(t, dy)
                    r0 = t * ROWT + lo + dy - 2
                    rhs = pb3[0:kmax, r0 : r0 + (hi - lo), dxa : dxa + W]
                    if CONV_DTYPE == "f32r":
                        rhs = rhs.bitcast(mybir.dt.float32r)
                    psr = pss[t][:].rearrange("p (r c) -> p r c", c=W)
                    nc.tensor.matmul(
                        psr[:, lo:hi, :], lhsT, rhs,
                        start=(i == 0), stop=(i == len(_TAPS) - 1),
                    )

            for t in range(NTILE):
                ps = pss[t]
                # rs = -GAMMA * R  (ScalarE)
                rs = wpool.tile([COUT, ROWT * W], f32, tag="rs")
                nc.scalar.activation(
                    rs[:], r3[:, t], mybir.ActivationFunctionType.Copy,
                    bias=0.0, scale=-GAMMA,
                )
                # U = (psum + bias) + rs ; S = U > 1
                u_sb = wpool.tile([COUT, ROWT * W], f32, tag="u_sb")
                nc.vector.scalar_tensor_tensor(
                    u_sb[:], ps[:], bias_sb[:], rs[:],
                    op0=mybir.AluOpType.add, op1=mybir.AluOpType.add,
                )
                s_sb = wpool.tile([COUT, ROWT * W], f32, tag="s_sb")
                nc.vector.tensor_scalar(
                    s_sb[:], u_sb[:], THR, None, op0=mybir.AluOpType.is_gt
                )
                nc.sync.dma_start(u3[:, t], u_sb[:])
                nc.sync.dma_start(s3[:, t], s_sb[:])

                # maxpool 2x2: row pairs then column pairs
                sq = s_sb[:].rearrange("p (r2 two c) -> p r2 two c", two=2, c=W)
                a_t = wpool.tile([128, (ROWT // 2) * W], f32, tag="a_t")
                a3 = a_t[:].rearrange("p (r c2 two) -> p r c2 two", two=2, c2=W // 2)
                nc.vector.tensor_tensor(
                    a_t[:], sq[:, :, 0, :], sq[:, :, 1, :], op=mybir.AluOpType.max
                )
                po5 = pooled[:].rearrange(
                    "p (tt pr pc b) -> p tt pr pc b",
                    tt=NTILE, pr=ROWT // 2, pc=W // 2,
                )
                nc.vector.tensor_tensor(
                    po5[:, t, :, :, b], a3[:, :, :, 0], a3[:, :, :, 1],
                    op=mybir.AluOpType.max,
                )

        # readout: accumulate over all 1024 p-chunks of K=128 (channels)
        # reuses a freed conv-psum slot (tag "ps") — keeps total PSUM at 8 banks
        ro_ps = pspool.tile([n_img, NOUT], f32, tag="ps", name="ro_ps")
        pog = pooled[:].rearrange("p (q b) -> p q b", b=n_img)
        wsg = ws_sb[:].rearrange("p (q o) -> p q o", o=NOUT)
        for p in range(NPOOL):
            nc.tensor.matmul(
                ro_ps[:], pog[:, p, :], wsg[:, p, :],
                start=(p == 0), stop=(p == NPOOL - 1),
            )
        ro_sb = cpool.tile([n_img, NOUT], f32, tag="rosb")
        nc.scalar.activation(
            ro_sb[:], ro_ps[:], mybir.ActivationFunctionType.Copy,
            bias=0.5, scale=0.5,
        )
        nc.sync.dma_start(ro_out.ap(), ro_sb[:])

    nc.compile()
    return nc


_NC_CACHE = {}


def _get_nc(n_img=BPC):
    key = (n_img, CONV_DTYPE)
    if key not in _NC_CACHE:
        _NC_CACHE[key] = _build(n_img)
    return _NC_CACHE[key]


def _host_prep(weights, bias, w_sign):
    sb_np = ml_dtypes.bfloat16 if CONV_DTYPE == "bf16" else np.float32
    wpre = (ALPHA * weights).astype(np.float32)  # [COUT, CIN, 5, 5]
    wconv = np.zeros((128, 15, 128), dtype=np.float32)
    for k, (dy, dxa, is_pair) in enumerate(_TAPS):
        wconv[0:CIN, k, :] = wpre[:, :, dy, dxa].T
        if is_pair:
            wconv[CIN:128, k, :] = wpre[:, :, dy, dxa + 1].T
    wconv = np.ascontiguousarray(wconv.reshape(128, 15 * 128).astype(sb_np))
    bias_v = np.ascontiguousarray(bias.reshape(COUT, 1).astype(np.float32))
    wsT = np.ascontiguousarray(
        w_sign.reshape(NOUT, 128, NPOOL).transpose(1, 2, 0)
        .reshape(128, NPOOL * NOUT).astype(ml_dtypes.bfloat16)
    )
    return wconv, bias_v, wsT


LAST_RESULT = None


def kernel(input_t=None, P=None, Q=None, R=None, weights=None, bias=None,
           w_sign=None, **_unused):
    global LAST_RESULT
    P = np.asarray(P, dtype=np.float32)
    Q = np.asarray(Q, dtype=np.float32)
    R = np.asarray(R, dtype=np.float32)
    weights = np.asarray(weights, dtype=np.float32)
    bias = np.asarray(bias, dtype=np.float32)
    w_sign = np.asarray(w_sign, dtype=np.float32)

    nc = _get_nc()
    wconv, bias_v, wsT = _host_prep(weights, bias, w_sign)

    in_maps = []
    for c in range(N_CORES):
        sl = slice(c * BPC, (c + 1) * BPC)
        in_maps.append({
            "p_in": np.ascontiguousarray(P[sl].reshape(BPC, CIN, HW)),
            "q_in": np.ascontiguousarray(Q[sl].reshape(BPC, CIN, HW)),
            "r_in": np.ascontiguousarray(R[sl].reshape(BPC, COUT, HW)),
            "wconv": wconv,
            "bias_v": bias_v,
            "wsT": wsT,
        })

    trace = bool(int(os.environ.get("LIF_TRACE", "0")))
    res = run_bass_kernel_spmd(nc, in_maps, core_ids=list(range(N_CORES)),
                               trace=trace)
    LAST_RESULT = res

    S = np.concatenate([r["s_out"] for r in res.results], axis=0)
    U = np.concatenate([r["u_out"] for r in res.results], axis=0)
    RO = np.concatenate([r["ro_out"] for r in res.results], axis=0)
    return (
        S.reshape(B, COUT, H, W),
        U.reshape(B, COUT, H, W),
        RO.reshape(B, NOUT),
    )
